# revision 4
# baseline (speedup 1.0000x reference)
"""Trainium2 Bass kernel for nn_BranchedNetwork (moe_routing).

Computation (reference):
    meas_embs = measurements @ W_meas + b_meas           [B, 512]
    embs      = concat([img_embs, meas_embs], axis=1)    [B, 1024]
    h_e       = relu(embs @ W1[e] + b1[e])               per expert e
    out_e     = h_e @ W2[e] + b2[e]
    p[i]      = out[command[i], i, 0]
    angle     = sigmoid(p) * 50 ; speed = clip(p, -1, 1)

Strategy:
  * Per-sample routing is done on the host: samples are grouped by
    command id, each group padded to a multiple of 8*128 rows and
    split evenly over the 8 cores (data parallel, weights replicated).
  * Only the selected expert runs per sample (4x less compute), and
    only column 0 of W2 is needed.
  * The measurement path is folded on the host:
      h_pre = img @ W1[e][:512] + meas @ (W_meas @ W1[e][512:])
              + (b_meas @ W1[e][512:] + b1[e])
    so the device contraction is K = 512 (img) + 8 (meas) + 1 (bias
    via a ones row) instead of 1024.
  * Device per 128-row tile: 5 PE matmuls (psum [128 rows, 512 hid]),
    ACT relu psum->sbuf, then one DVE tensor_tensor_reduce computing
    p = relu_h @ w2col + b2 as a fused multiply + free-dim reduction.
    Final sigmoid/scale/clip on [128, n_tiles] at full lane width.
"""

import os
import sys
import types

import numpy as np

if "/opt/trn_rl_repo" not in sys.path and not any(
    p.endswith("trn_rl_repo") for p in sys.path
):
    sys.path.insert(0, "/opt/trn_rl_repo")

B = 16384
EMB = 512
NUM_COMMANDS = 4
NUM_MEAS = 8
NCORES = 8
P = 128

# matmul dtype mode: "f32" (exact, 4 cyc/row), "f32r" (full speed,
# reduced internal precision), "bf16" (full speed + half DMA traffic)
MODE = os.environ.get("KERNEL_MM_MODE", "f32r")

_CACHE = {}


def _install_ntff_shim():
    """Recreate antenv.axon_hooks so trace=True works if requested."""
    if "antenv.axon_hooks" in sys.modules:
        return
    try:
        import antenv

        mod = types.ModuleType("antenv.axon_hooks")
        mod._hook = None
        mod.set_axon_ntff_profile_hook = lambda h: setattr(mod, "_hook", h)
        mod.get_axon_ntff_profile_hook = lambda: mod._hook
        sys.modules["antenv.axon_hooks"] = mod
        antenv.axon_hooks = mod
        from trn_agent_boot.trn_boot import _ntff_profile_via_ctypes

        mod.set_axon_ntff_profile_hook(
            _ntff_profile_via_ctypes("/opt/axon/libaxon_pjrt.so")
        )
    except Exception:
        pass


def _split_excess_waits(nc, max_waits=1):
    """The walrus in this container rejects instructions with more than
    one embedded sync-wait command. Waits execute in order on the
    issuing engine, so hoisting the excess onto preceding NOPs on the
    same engine is semantically identical."""
    from concourse import mybir

    n_split = 0
    for f in nc.m.functions:
        for bb in f.blocks:
            insts = list(bb.instructions)
            new_insts = []
            changed = False
            for inst in insts:
                si = inst.sync_info
                if si is not None and si.on_wait and len(si.on_wait) > max_waits:
                    waits = list(si.on_wait)
                    extra, keep = waits[:-max_waits], waits[-max_waits:]
                    while extra:
                        chunk, extra = extra[:max_waits], extra[max_waits:]
                        n_split += 1
                        nop = mybir.InstNoOp(
                            name=f"waitsplit_{n_split}_{inst.name}",
                            engine=inst.engine,
                            ins=[],
                            outs=[],
                            sync_info=mybir.SyncInfo(on_wait=chunk, on_update=[]),
                        )
                        new_insts.append(nop)
                    si.on_wait = keep
                    changed = True
                new_insts.append(inst)
            if changed:
                bb.instructions.clear()
                for i in new_insts:
                    bb.instructions.append(i)
    return n_split


def _np_sto_dtype(mode):
    if mode == "bf16":
        import ml_dtypes

        return ml_dtypes.bfloat16
    return np.float32


def _route(command):
    """Group sample indices by expert, pad each group to a multiple of
    8*128 and split evenly across cores.

    Returns caps [E] (rows per core per expert) and I [NCORES, R] row
    index arrays (R = sum(caps))."""
    caps = []
    parts = []  # per expert: [NCORES, cap_e] padded index array
    for e in range(NUM_COMMANDS):
        idx = np.nonzero(command == e)[0].astype(np.int64)
        n = len(idx)
        cap = int(np.ceil(n / (NCORES * P))) * P if n else 0
        caps.append(cap)
        if cap == 0:
            parts.append(np.zeros((NCORES, 0), np.int64))
            continue
        pad = NCORES * cap - n
        idx_pad = np.concatenate([idx, np.full(pad, idx[-1], np.int64)])
        parts.append(idx_pad.reshape(NCORES, cap))
    I = [np.concatenate([parts[e][k] for e in range(NUM_COMMANDS)]) for k in range(NCORES)]
    return caps, np.stack(I)


def _build_program(R, caps, b2c, mode):
    from contextlib import ExitStack

    import concourse.bass as bass
    import concourse.tile as tile
    from concourse import mybir

    f32 = mybir.dt.float32
    # matmul-operand dtype (the whole producer chain must carry it for
    # the fp32r BIR verifier) and elementwise/storage dtype
    if mode == "bf16":
        MMD = mybir.dt.bfloat16
        STO = mybir.dt.bfloat16
    elif mode == "f32r":
        MMD = mybir.dt.float32r
        STO = f32
    else:
        MMD = f32
        STO = f32
    T = R // P

    nc = bass.Bass()
    imgT_d = nc.declare_dram_parameter("imgT", [EMB, R], MMD, isOutput=False)
    measT_d = nc.declare_dram_parameter("measAug", [NUM_MEAS + 1, R], MMD, isOutput=False)
    A_d = nc.declare_dram_parameter("A", [NUM_COMMANDS, EMB, EMB], MMD, isOutput=False)
    WfAug_d = nc.declare_dram_parameter(
        "WfAug", [NUM_COMMANDS, NUM_MEAS + 1, EMB], MMD, isOutput=False
    )
    w2_d = nc.declare_dram_parameter("w2rep", [NUM_COMMANDS, P, EMB], STO, isOutput=False)
    b2tail_d = nc.declare_dram_parameter("b2tail", [P, T], f32, isOutput=False)
    outp_d = nc.declare_dram_parameter("outp", [P, 2, T], f32, isOutput=True)

    with tile.TileContext(nc) as tc:
        with ExitStack() as ctx:
            const_pool = ctx.enter_context(tc.tile_pool(name="const", bufs=1))
            w_pool = ctx.enter_context(tc.tile_pool(name="w", bufs=4))
            img_pool = ctx.enter_context(tc.tile_pool(name="img", bufs=4))
            relu_pool = ctx.enter_context(tc.tile_pool(name="relu", bufs=4))
            junk_pool = ctx.enter_context(tc.tile_pool(name="junk", bufs=2))
            out_pool = ctx.enter_context(tc.tile_pool(name="out", bufs=1))
            ps_pool = ctx.enter_context(tc.tile_pool(name="ps", bufs=4, space="PSUM"))

            measT_sb = const_pool.tile([NUM_MEAS + 1, R], MMD)
            nc.sync.dma_start(measT_sb[:], measT_d[:])
            WfAug_sb = const_pool.tile([NUM_MEAS + 1, NUM_COMMANDS, EMB], MMD)
            nc.sync.dma_start(WfAug_sb[:], WfAug_d[:].rearrange("e k m -> k e m"))
            w2_sb = const_pool.tile([P, NUM_COMMANDS, EMB], STO)
            nc.sync.dma_start(w2_sb[:], w2_d[:].rearrange("e p m -> p e m"))
            b2tail_sb = const_pool.tile([P, T], f32)
            nc.sync.dma_start(b2tail_sb[:], b2tail_d[:])

            p_all = out_pool.tile([P, T], f32)

            A_sb = {}
            img_sb = {}
            for e, cap in enumerate(caps):
                if cap == 0:
                    continue
                A_sb[e] = w_pool.tile([P, 4, EMB], MMD, tag="A", name=f"A_sb_{e}")
                nc.sync.dma_start(
                    A_sb[e][:], A_d[e].rearrange("(o p) m -> p o m", p=P)
                )
                img_sb[e] = img_pool.tile([P, 4, cap], MMD, tag="img", name=f"img_sb_{e}")
                off = sum(caps[:e])
                nc.sync.dma_start(
                    img_sb[e][:],
                    imgT_d[:, off : off + cap].rearrange("(o p) r -> p o r", p=P),
                )

            g = 0
            for e, cap in enumerate(caps):
                off = sum(caps[:e])
                for r in range(cap // P):
                    ps = ps_pool.tile([P, EMB], f32, tag="h")
                    for ko in range(4):
                        nc.tensor.matmul(
                            ps[:],
                            lhsT=img_sb[e][:, ko, r * P : (r + 1) * P],
                            rhs=A_sb[e][:, ko, :],
                            start=(ko == 0),
                            stop=False,
                        )
                    col = off + r * P
                    nc.tensor.matmul(
                        ps[:],
                        lhsT=measT_sb[:, col : col + P],
                        rhs=WfAug_sb[:, e, :],
                        start=False,
                        stop=True,
                    )
                    relu_t = relu_pool.tile([P, EMB], STO, tag="relu")
                    nc.scalar.activation(
                        relu_t[:], ps[:], mybir.ActivationFunctionType.Relu
                    )
                    junk = junk_pool.tile([P, EMB], STO, tag="junk")
                    nc.vector.scalar_tensor_tensor(
                        out=junk[:],
                        in0=relu_t[:],
                        scalar=1.0,
                        in1=w2_sb[:, e, :],
                        op0=mybir.AluOpType.mult,
                        op1=mybir.AluOpType.mult,
                        accum_out=p_all[:, g : g + 1],
                    )
                    g += 1

            q = out_pool.tile([P, T], f32)
            nc.vector.tensor_add(q[:], p_all[:], b2tail_sb[:])
            sig = out_pool.tile([P, T], f32)
            nc.scalar.activation(
                sig[:], q[:], mybir.ActivationFunctionType.Sigmoid
            )
            outs = out_pool.tile([P, 2, T], f32)
            nc.vector.tensor_scalar_mul(outs[:, 0, :], sig[:], 50.0)
            nc.vector.tensor_scalar(
                outs[:, 1, :],
                q[:],
                1.0,
                -1.0,
                mybir.AluOpType.min,
                mybir.AluOpType.max,
            )
            nc.sync.dma_start(outp_d[:], outs[:])

    _split_excess_waits(nc)
    return nc


def _prepare(inputs, mode):
    img_embs = np.asarray(inputs["img_embs"], np.float32)
    measurements = np.asarray(inputs["measurements"], np.float32)
    command = np.asarray(inputs["command"])
    W_meas = np.asarray(inputs["W_meas"], np.float32)
    b_meas = np.asarray(inputs["b_meas"], np.float32)
    W1 = np.asarray(inputs["W1"], np.float32)
    b1 = np.asarray(inputs["b1"], np.float32)
    W2 = np.asarray(inputs["W2"], np.float32)
    b2 = np.asarray(inputs["b2"], np.float32)

    sto = _np_sto_dtype(mode)
    caps, I = _route(command)
    R = int(sum(caps))

    # fold measurement path (float64 for the host-side precompute)
    W1h = W1[:, EMB:, :].astype(np.float64)
    Wf = np.einsum("md,edh->emh", W_meas.astype(np.float64), W1h)
    b_eff = np.einsum("d,edh->eh", b_meas.astype(np.float64), W1h) + b1
    WfAug = np.concatenate([Wf, b_eff[:, None, :]], axis=1).astype(sto)  # [E,9,H]
    A = np.ascontiguousarray(W1[:, :EMB, :]).astype(sto)  # [E,512,512]
    w2c = W2[:, :, 0]
    w2rep = np.ascontiguousarray(
        np.broadcast_to(w2c[:, None, :], (NUM_COMMANDS, P, EMB))
    ).astype(sto)
    b2c = [float(x) for x in b2[:, 0]]

    T = R // P
    col_expert = np.concatenate(
        [np.full(caps[e] // P, e, np.int64) for e in range(NUM_COMMANDS)]
    )
    b2tail = np.broadcast_to(
        np.array([b2c[e] for e in col_expert], np.float32)[None, :], (P, T)
    ).copy()

    imgT = img_embs.T  # [512, B]
    measT = measurements.T  # [8, B]
    in_maps = []
    for k in range(NCORES):
        Ik = I[k]
        imgT_k = np.ascontiguousarray(imgT[:, Ik]).astype(sto)
        measAug_k = np.concatenate(
            [measT[:, Ik], np.ones((1, R), np.float32)], axis=0
        ).astype(sto)
        in_maps.append(
            {
                "imgT": imgT_k,
                "measAug": measAug_k,
                "A": A,
                "WfAug": WfAug,
                "w2rep": w2rep,
                "b2tail": b2tail,
            }
        )
    return in_maps, I, R, caps, b2c


def _run(inputs, mode=None, trace=False):
    """Returns ((angle, speed), BassKernelResults)."""
    mode = mode or MODE
    _install_ntff_shim()
    from concourse.bass_utils import run_bass_kernel_spmd

    in_maps, I, R, caps, b2c = _prepare(inputs, mode)
    key = (R, tuple(caps), mode, tuple(np.float32(b) for b in b2c))
    if key not in _CACHE:
        _CACHE[key] = _build_program(R, caps, b2c, mode)
    nc = _CACHE[key]

    res = run_bass_kernel_spmd(
        nc, in_maps, core_ids=list(range(NCORES)), trace=trace
    )

    angle = np.zeros(B, np.float32)
    speed = np.zeros(B, np.float32)
    for k in range(NCORES):
        outp = res.results[k]["outp"]  # [128, 2, T]
        Ik = I[k]
        angle[Ik] = outp[:, 0, :].T.reshape(R)
        speed[Ik] = outp[:, 1, :].T.reshape(R)
    return (angle, speed), res


def kernel(**inputs):
    out, _ = _run(inputs)
    return out


# revision 5
# speedup vs baseline: 1.5118x; 1.5118x over previous
"""Trainium2 Bass kernel for nn_BranchedNetwork (moe_routing).

Computation (reference):
    meas_embs = measurements @ W_meas + b_meas           [B, 512]
    embs      = concat([img_embs, meas_embs], axis=1)    [B, 1024]
    h_e       = relu(embs @ W1[e] + b1[e])               per expert e
    out_e     = h_e @ W2[e] + b2[e]
    p[i]      = out[command[i], i, 0]
    angle     = sigmoid(p) * 50 ; speed = clip(p, -1, 1)

Strategy:
  * Per-sample routing is done on the host: samples are grouped by
    command id, each group padded to a multiple of 8*128 rows and
    split evenly over the 8 cores (data parallel, weights replicated).
  * Only the selected expert runs per sample (4x less compute), and
    only column 0 of W2 is needed.
  * The measurement path is folded on the host:
      h_pre = img @ W1[e][:512] + meas @ (W_meas @ W1[e][512:])
              + (b_meas @ W1[e][512:] + b1[e])
    so the device contraction is K = 512 (img) + 8 (meas) + 1 (bias
    via a ones row) instead of 1024.
  * Device per 128-row tile: 5 PE matmuls (psum [128 rows, 512 hid]),
    ACT relu psum->sbuf, then one DVE tensor_tensor_reduce computing
    p = relu_h @ w2col + b2 as a fused multiply + free-dim reduction.
    Final sigmoid/scale/clip on [128, n_tiles] at full lane width.
"""

import os
import sys
import types

import numpy as np

if "/opt/trn_rl_repo" not in sys.path and not any(
    p.endswith("trn_rl_repo") for p in sys.path
):
    sys.path.insert(0, "/opt/trn_rl_repo")

B = 16384
EMB = 512
NUM_COMMANDS = 4
NUM_MEAS = 8
NCORES = 8
P = 128

# matmul dtype mode: "f32" (exact, 4 cyc/row), "f32r" (full speed,
# reduced internal precision), "bf16" (full speed + half DMA traffic)
MODE = os.environ.get("KERNEL_MM_MODE", "f32r")

_CACHE = {}


def _install_ntff_shim():
    """Recreate antenv.axon_hooks so trace=True works if requested."""
    if "antenv.axon_hooks" in sys.modules:
        return
    try:
        import antenv

        mod = types.ModuleType("antenv.axon_hooks")
        mod._hook = None
        mod.set_axon_ntff_profile_hook = lambda h: setattr(mod, "_hook", h)
        mod.get_axon_ntff_profile_hook = lambda: mod._hook
        sys.modules["antenv.axon_hooks"] = mod
        antenv.axon_hooks = mod
        from trn_agent_boot.trn_boot import _ntff_profile_via_ctypes

        mod.set_axon_ntff_profile_hook(
            _ntff_profile_via_ctypes("/opt/axon/libaxon_pjrt.so")
        )
    except Exception:
        pass


def _split_excess_waits(nc, max_waits=1):
    """The walrus in this container rejects instructions with more than
    one embedded sync-wait command. Waits execute in order on the
    issuing engine, so hoisting the excess onto preceding NOPs on the
    same engine is semantically identical."""
    from concourse import mybir

    n_split = 0
    for f in nc.m.functions:
        for bb in f.blocks:
            insts = list(bb.instructions)
            new_insts = []
            changed = False
            for inst in insts:
                si = inst.sync_info
                if si is not None and si.on_wait and len(si.on_wait) > max_waits:
                    waits = list(si.on_wait)
                    extra, keep = waits[:-max_waits], waits[-max_waits:]
                    while extra:
                        chunk, extra = extra[:max_waits], extra[max_waits:]
                        n_split += 1
                        nop = mybir.InstNoOp(
                            name=f"waitsplit_{n_split}_{inst.name}",
                            engine=inst.engine,
                            ins=[],
                            outs=[],
                            sync_info=mybir.SyncInfo(on_wait=chunk, on_update=[]),
                        )
                        new_insts.append(nop)
                    si.on_wait = keep
                    changed = True
                new_insts.append(inst)
            if changed:
                bb.instructions.clear()
                for i in new_insts:
                    bb.instructions.append(i)
    return n_split


def _np_sto_dtype(mode):
    if mode == "bf16":
        import ml_dtypes

        return ml_dtypes.bfloat16
    return np.float32


def _route(command):
    """Group sample indices by expert, pad each group to a multiple of
    8*128 and split evenly across cores.

    Returns caps [E] (rows per core per expert) and I [NCORES, R] row
    index arrays (R = sum(caps))."""
    caps = []
    parts = []  # per expert: [NCORES, cap_e] padded index array
    for e in range(NUM_COMMANDS):
        idx = np.nonzero(command == e)[0].astype(np.int64)
        n = len(idx)
        cap = int(np.ceil(n / (NCORES * P))) * P if n else 0
        caps.append(cap)
        if cap == 0:
            parts.append(np.zeros((NCORES, 0), np.int64))
            continue
        pad = NCORES * cap - n
        idx_pad = np.concatenate([idx, np.full(pad, idx[-1], np.int64)])
        parts.append(idx_pad.reshape(NCORES, cap))
    I = [np.concatenate([parts[e][k] for e in range(NUM_COMMANDS)]) for k in range(NCORES)]
    return caps, np.stack(I)


def _build_program(R, caps, b2c, mode):
    from contextlib import ExitStack

    import concourse.bass as bass
    import concourse.tile as tile
    from concourse import mybir

    f32 = mybir.dt.float32
    # matmul-operand dtype (the whole producer chain must carry it for
    # the fp32r BIR verifier) and elementwise/storage dtype
    if mode == "bf16":
        MMD = mybir.dt.bfloat16
        STO = mybir.dt.bfloat16
    elif mode == "f32r":
        MMD = mybir.dt.float32r
        STO = f32
    else:
        MMD = f32
        STO = f32
    T = R // P

    nc = bass.Bass()
    # all arrays are PRE-TILED on the host so every DMA is a dense
    # [partition, contiguous-bytes] copy (cheap descriptor generation)
    imgT_d = nc.declare_dram_parameter("img_pre", [P, 4 * R], MMD, isOutput=False)
    measT_d = nc.declare_dram_parameter("measAug", [NUM_MEAS + 1, R], MMD, isOutput=False)
    A_d = nc.declare_dram_parameter("A_pre", [NUM_COMMANDS, P, 4 * EMB], MMD, isOutput=False)
    WfAug_d = nc.declare_dram_parameter(
        "WfAug_pre", [NUM_MEAS + 1, NUM_COMMANDS, EMB], MMD, isOutput=False
    )
    w2_d = nc.declare_dram_parameter("w2_pre", [P, NUM_COMMANDS, EMB], STO, isOutput=False)
    b2tail_d = nc.declare_dram_parameter("b2tail", [P, T], f32, isOutput=False)
    outp_d = nc.declare_dram_parameter("outp", [P, 2, T], f32, isOutput=True)

    with tile.TileContext(nc) as tc:
        with ExitStack() as ctx:
            const_pool = ctx.enter_context(tc.tile_pool(name="const", bufs=1))
            w_pool = ctx.enter_context(tc.tile_pool(name="w", bufs=4))
            img_pool = ctx.enter_context(tc.tile_pool(name="img", bufs=4))
            relu_pool = ctx.enter_context(tc.tile_pool(name="relu", bufs=4))
            junk_pool = ctx.enter_context(tc.tile_pool(name="junk", bufs=2))
            out_pool = ctx.enter_context(tc.tile_pool(name="out", bufs=1))
            ps_pool = ctx.enter_context(tc.tile_pool(name="ps", bufs=4, space="PSUM"))

            measT_sb = const_pool.tile([NUM_MEAS + 1, R], MMD)
            nc.sync.dma_start(measT_sb[:], measT_d[:])
            WfAug_sb = const_pool.tile([NUM_MEAS + 1, NUM_COMMANDS, EMB], MMD)
            nc.gpsimd.dma_start(WfAug_sb[:], WfAug_d[:])
            w2_sb = const_pool.tile([P, NUM_COMMANDS, EMB], STO)
            nc.gpsimd.dma_start(w2_sb[:], w2_d[:])
            b2tail_sb = const_pool.tile([P, T], f32)
            nc.sync.dma_start(b2tail_sb[:], b2tail_d[:])

            p_all = out_pool.tile([P, T], f32)

            A_sb = {}
            img_sb = {}
            for e, cap in enumerate(caps):
                if cap == 0:
                    continue
                A_sb[e] = w_pool.tile([P, 4, EMB], MMD, tag="A", name=f"A_sb_{e}")
                nc.gpsimd.dma_start(
                    A_sb[e][:], A_d[e].rearrange("p (o m) -> p o m", o=4)
                )
                img_sb[e] = img_pool.tile([P, 4, cap], MMD, tag="img", name=f"img_sb_{e}")
                base = 4 * sum(caps[:e])
                nc.sync.dma_start(
                    img_sb[e][:],
                    imgT_d[:, base : base + 4 * cap].rearrange("p (o r) -> p o r", o=4),
                )

            g = 0
            for e, cap in enumerate(caps):
                off = sum(caps[:e])
                for r in range(cap // P):
                    ps = ps_pool.tile([P, EMB], f32, tag="h")
                    for ko in range(4):
                        nc.tensor.matmul(
                            ps[:],
                            lhsT=img_sb[e][:, ko, r * P : (r + 1) * P],
                            rhs=A_sb[e][:, ko, :],
                            start=(ko == 0),
                            stop=False,
                        )
                    col = off + r * P
                    nc.tensor.matmul(
                        ps[:],
                        lhsT=measT_sb[:, col : col + P],
                        rhs=WfAug_sb[:, e, :],
                        start=False,
                        stop=True,
                    )
                    relu_t = relu_pool.tile([P, EMB], STO, tag="relu")
                    nc.scalar.activation(
                        relu_t[:], ps[:], mybir.ActivationFunctionType.Relu
                    )
                    junk = junk_pool.tile([P, EMB], STO, tag="junk")
                    nc.vector.scalar_tensor_tensor(
                        out=junk[:],
                        in0=relu_t[:],
                        scalar=1.0,
                        in1=w2_sb[:, e, :],
                        op0=mybir.AluOpType.mult,
                        op1=mybir.AluOpType.mult,
                        accum_out=p_all[:, g : g + 1],
                    )
                    g += 1

            q = out_pool.tile([P, T], f32)
            nc.vector.tensor_add(q[:], p_all[:], b2tail_sb[:])
            sig = out_pool.tile([P, T], f32)
            nc.scalar.activation(
                sig[:], q[:], mybir.ActivationFunctionType.Sigmoid
            )
            outs = out_pool.tile([P, 2, T], f32)
            nc.vector.tensor_scalar_mul(outs[:, 0, :], sig[:], 50.0)
            nc.vector.tensor_scalar(
                outs[:, 1, :],
                q[:],
                1.0,
                -1.0,
                mybir.AluOpType.min,
                mybir.AluOpType.max,
            )
            nc.sync.dma_start(outp_d[:], outs[:])

    _split_excess_waits(nc)
    return nc


def _prepare(inputs, mode):
    img_embs = np.asarray(inputs["img_embs"], np.float32)
    measurements = np.asarray(inputs["measurements"], np.float32)
    command = np.asarray(inputs["command"])
    W_meas = np.asarray(inputs["W_meas"], np.float32)
    b_meas = np.asarray(inputs["b_meas"], np.float32)
    W1 = np.asarray(inputs["W1"], np.float32)
    b1 = np.asarray(inputs["b1"], np.float32)
    W2 = np.asarray(inputs["W2"], np.float32)
    b2 = np.asarray(inputs["b2"], np.float32)

    sto = _np_sto_dtype(mode)
    caps, I = _route(command)
    R = int(sum(caps))

    # fold measurement path (float64 for the host-side precompute)
    W1h = W1[:, EMB:, :].astype(np.float64)
    Wf = np.einsum("md,edh->emh", W_meas.astype(np.float64), W1h)
    b_eff = np.einsum("d,edh->eh", b_meas.astype(np.float64), W1h) + b1
    WfAug = np.concatenate([Wf, b_eff[:, None, :]], axis=1).astype(sto)  # [E,9,H]
    A = np.ascontiguousarray(W1[:, :EMB, :]).astype(sto)  # [E,512,512]
    w2c = W2[:, :, 0]
    w2rep = np.ascontiguousarray(
        np.broadcast_to(w2c[:, None, :], (NUM_COMMANDS, P, EMB))
    ).astype(sto)
    b2c = [float(x) for x in b2[:, 0]]

    T = R // P
    col_expert = np.concatenate(
        [np.full(caps[e] // P, e, np.int64) for e in range(NUM_COMMANDS)]
    )
    b2tail = np.broadcast_to(
        np.array([b2c[e] for e in col_expert], np.float32)[None, :], (P, T)
    ).copy()

    # pre-tiled shared weights: every device DMA is a dense 2D copy
    A_pre = np.ascontiguousarray(
        A.reshape(NUM_COMMANDS, 4, P, EMB).transpose(0, 2, 1, 3).reshape(
            NUM_COMMANDS, P, 4 * EMB
        )
    )
    WfAug_pre = np.ascontiguousarray(WfAug.transpose(1, 0, 2))  # [9, E, 512]
    w2_pre = np.ascontiguousarray(w2rep.transpose(1, 0, 2))  # [128, E, 512]

    imgT = img_embs.T.astype(sto)  # [512, B] cast once
    measT = measurements.T  # [8, B]
    ones_row = np.ones((1, R), np.float32).astype(sto)
    in_maps = []
    for k in range(NCORES):
        Ik = I[k]
        imgT_k = imgT[:, Ik].reshape(4, P, R)  # [o, p, r]
        img_pre = np.concatenate(
            [
                imgT_k[:, :, sum(caps[:e]) : sum(caps[: e + 1])]
                .transpose(1, 0, 2)
                .reshape(P, 4 * caps[e])
                for e in range(NUM_COMMANDS)
                if caps[e]
            ],
            axis=1,
        )
        measAug_k = np.concatenate(
            [measT[:, Ik].astype(sto), ones_row], axis=0
        )
        in_maps.append(
            {
                "img_pre": np.ascontiguousarray(img_pre),
                "measAug": measAug_k,
                "A_pre": A_pre,
                "WfAug_pre": WfAug_pre,
                "w2_pre": w2_pre,
                "b2tail": b2tail,
            }
        )
    return in_maps, I, R, caps, b2c


def _run(inputs, mode=None, trace=False):
    """Returns ((angle, speed), BassKernelResults)."""
    mode = mode or MODE
    _install_ntff_shim()
    from concourse.bass_utils import run_bass_kernel_spmd

    in_maps, I, R, caps, b2c = _prepare(inputs, mode)
    key = (R, tuple(caps), mode, tuple(np.float32(b) for b in b2c))
    if key not in _CACHE:
        _CACHE[key] = _build_program(R, caps, b2c, mode)
    nc = _CACHE[key]

    res = run_bass_kernel_spmd(
        nc, in_maps, core_ids=list(range(NCORES)), trace=trace
    )

    angle = np.zeros(B, np.float32)
    speed = np.zeros(B, np.float32)
    for k in range(NCORES):
        outp = res.results[k]["outp"]  # [128, 2, T]
        Ik = I[k]
        angle[Ik] = outp[:, 0, :].T.reshape(R)
        speed[Ik] = outp[:, 1, :].T.reshape(R)
    return (angle, speed), res


def kernel(**inputs):
    out, _ = _run(inputs)
    return out


# revision 6
# speedup vs baseline: 1.5359x; 1.0159x over previous
"""Trainium2 Bass kernel for nn_BranchedNetwork (moe_routing).

Computation (reference):
    meas_embs = measurements @ W_meas + b_meas           [B, 512]
    embs      = concat([img_embs, meas_embs], axis=1)    [B, 1024]
    h_e       = relu(embs @ W1[e] + b1[e])               per expert e
    out_e     = h_e @ W2[e] + b2[e]
    p[i]      = out[command[i], i, 0]
    angle     = sigmoid(p) * 50 ; speed = clip(p, -1, 1)

Strategy:
  * Per-sample routing is done on the host: samples are grouped by
    command id, each group padded to a multiple of 8*128 rows and
    split evenly over the 8 cores (data parallel, weights replicated).
  * Only the selected expert runs per sample (4x less compute), and
    only column 0 of W2 is needed.
  * The measurement path is folded on the host:
      h_pre = img @ W1[e][:512] + meas @ (W_meas @ W1[e][512:])
              + (b_meas @ W1[e][512:] + b1[e])
    so the device contraction is K = 512 (img) + 8 (meas) + 1 (bias
    via a ones row) instead of 1024.
  * Device per 128-row tile: 5 PE matmuls (psum [128 rows, 512 hid]),
    ACT relu psum->sbuf, then one DVE tensor_tensor_reduce computing
    p = relu_h @ w2col + b2 as a fused multiply + free-dim reduction.
    Final sigmoid/scale/clip on [128, n_tiles] at full lane width.
"""

import os
import sys
import types

import numpy as np

if "/opt/trn_rl_repo" not in sys.path and not any(
    p.endswith("trn_rl_repo") for p in sys.path
):
    sys.path.insert(0, "/opt/trn_rl_repo")

B = 16384
EMB = 512
NUM_COMMANDS = 4
NUM_MEAS = 8
NCORES = 8
P = 128

# matmul dtype mode: "f32" (exact, 4 cyc/row), "f32r" (full speed,
# reduced internal precision), "bf16" (full speed + half DMA traffic)
MODE = os.environ.get("KERNEL_MM_MODE", "f32r")

_CACHE = {}


def _install_ntff_shim():
    """Recreate antenv.axon_hooks so trace=True works if requested."""
    if "antenv.axon_hooks" in sys.modules:
        return
    try:
        import antenv

        mod = types.ModuleType("antenv.axon_hooks")
        mod._hook = None
        mod.set_axon_ntff_profile_hook = lambda h: setattr(mod, "_hook", h)
        mod.get_axon_ntff_profile_hook = lambda: mod._hook
        sys.modules["antenv.axon_hooks"] = mod
        antenv.axon_hooks = mod
        from trn_agent_boot.trn_boot import _ntff_profile_via_ctypes

        mod.set_axon_ntff_profile_hook(
            _ntff_profile_via_ctypes("/opt/axon/libaxon_pjrt.so")
        )
    except Exception:
        pass


def _split_excess_waits(nc, max_waits=1):
    """The walrus in this container rejects instructions with more than
    one embedded sync-wait command. Waits execute in order on the
    issuing engine, so hoisting the excess onto preceding NOPs on the
    same engine is semantically identical."""
    from concourse import mybir

    n_split = 0
    for f in nc.m.functions:
        for bb in f.blocks:
            insts = list(bb.instructions)
            new_insts = []
            changed = False
            for inst in insts:
                si = inst.sync_info
                if si is not None and si.on_wait and len(si.on_wait) > max_waits:
                    waits = list(si.on_wait)
                    extra, keep = waits[:-max_waits], waits[-max_waits:]
                    while extra:
                        chunk, extra = extra[:max_waits], extra[max_waits:]
                        n_split += 1
                        nop = mybir.InstNoOp(
                            name=f"waitsplit_{n_split}_{inst.name}",
                            engine=inst.engine,
                            ins=[],
                            outs=[],
                            sync_info=mybir.SyncInfo(on_wait=chunk, on_update=[]),
                        )
                        new_insts.append(nop)
                    si.on_wait = keep
                    changed = True
                new_insts.append(inst)
            if changed:
                bb.instructions.clear()
                for i in new_insts:
                    bb.instructions.append(i)
    return n_split


def _strip_tail(nc):
    """Remove the end-of-kernel barrier/sem-reset tail.

    The runtime clears semaphores in its own exec preamble, and every
    engine's results flow into the output DMA via data-dependency
    semaphores, so the only thing that must remain is the sync-engine
    DRAIN that flushes the output DMA queue."""
    from concourse import mybir

    f = nc.m.functions[0]
    bb = f.blocks[-1]
    insts = list(bb.instructions)
    idx = None
    for i, inst in enumerate(insts):
        if isinstance(inst, mybir.InstDrain) and inst.engine == mybir.EngineType.SP:
            idx = i
            break
    if idx is None:
        return 0
    kept = insts[: idx + 1]
    drain = kept[-1]
    if drain.sync_info is not None:
        drain.sync_info.on_wait = []
    removed = len(insts) - len(kept)
    bb.instructions.clear()
    for i in kept:
        bb.instructions.append(i)
    return removed


def _np_sto_dtype(mode):
    if mode == "bf16":
        import ml_dtypes

        return ml_dtypes.bfloat16
    return np.float32


def _route(command):
    """Group sample indices by expert, pad each group to a multiple of
    8*128 and split evenly across cores.

    Returns caps [E] (rows per core per expert) and I [NCORES, R] row
    index arrays (R = sum(caps))."""
    caps = []
    parts = []  # per expert: [NCORES, cap_e] padded index array
    for e in range(NUM_COMMANDS):
        idx = np.nonzero(command == e)[0].astype(np.int64)
        n = len(idx)
        cap = int(np.ceil(n / (NCORES * P))) * P if n else 0
        caps.append(cap)
        if cap == 0:
            parts.append(np.zeros((NCORES, 0), np.int64))
            continue
        pad = NCORES * cap - n
        idx_pad = np.concatenate([idx, np.full(pad, idx[-1], np.int64)])
        parts.append(idx_pad.reshape(NCORES, cap))
    I = [np.concatenate([parts[e][k] for e in range(NUM_COMMANDS)]) for k in range(NCORES)]
    return caps, np.stack(I)


def _build_program(R, caps, b2c, mode):
    from contextlib import ExitStack

    import concourse.bass as bass
    import concourse.tile as tile
    from concourse import mybir

    f32 = mybir.dt.float32
    # matmul-operand dtype (the whole producer chain must carry it for
    # the fp32r BIR verifier) and elementwise/storage dtype
    if mode == "bf16":
        MMD = mybir.dt.bfloat16
        STO = mybir.dt.bfloat16
    elif mode == "f32r":
        MMD = mybir.dt.float32r
        STO = f32
    else:
        MMD = f32
        STO = f32
    T = R // P

    nc = bass.Bass()
    # all arrays are PRE-TILED on the host so every DMA is a dense
    # [partition, contiguous-bytes] copy (cheap descriptor generation)
    imgT_d = nc.declare_dram_parameter("img_pre", [P, 4 * R], MMD, isOutput=False)
    measT_d = nc.declare_dram_parameter("measAug", [NUM_MEAS + 1, R], MMD, isOutput=False)
    A_d = nc.declare_dram_parameter("A_pre", [NUM_COMMANDS, P, 4 * EMB], MMD, isOutput=False)
    WfAug_d = nc.declare_dram_parameter(
        "WfAug_pre", [NUM_MEAS + 1, NUM_COMMANDS, EMB], MMD, isOutput=False
    )
    w2_d = nc.declare_dram_parameter("w2c", [NUM_COMMANDS, EMB], MMD, isOutput=False)
    b2tail_d = nc.declare_dram_parameter("b2tail", [P, T], f32, isOutput=False)
    outp_d = nc.declare_dram_parameter("outp", [P, 2, T], f32, isOutput=True)

    with tile.TileContext(nc) as tc:
        with ExitStack() as ctx:
            const_pool = ctx.enter_context(tc.tile_pool(name="const", bufs=1))
            w_pool = ctx.enter_context(tc.tile_pool(name="w", bufs=4))
            w2_pool = ctx.enter_context(tc.tile_pool(name="w2", bufs=4))
            img_pool = ctx.enter_context(tc.tile_pool(name="img", bufs=4))
            relu_pool = ctx.enter_context(tc.tile_pool(name="relu", bufs=6))
            junk_pool = ctx.enter_context(tc.tile_pool(name="junk", bufs=3))
            out_pool = ctx.enter_context(tc.tile_pool(name="out", bufs=1))
            ps_pool = ctx.enter_context(tc.tile_pool(name="ps", bufs=6, space="PSUM"))

            # greedy least-loaded DMA queue assignment over the three
            # DMA-capable engines (SP + ACT hwdge, Pool swdge)
            dma_engines = [nc.sync, nc.scalar, nc.gpsimd]
            dma_load = [0, 0, 0]

            def dma(dst, src, nbytes):
                qi = dma_load.index(min(dma_load))
                dma_load[qi] += nbytes
                dma_engines[qi].dma_start(dst, src)

            esz = 2 if mode == "bf16" else 4
            measT_sb = const_pool.tile([NUM_MEAS + 1, R], MMD)
            dma(measT_sb[:], measT_d[:], 9 * R * esz)
            WfAug_sb = const_pool.tile([NUM_MEAS + 1, NUM_COMMANDS, EMB], MMD)
            dma(WfAug_sb[:], WfAug_d[:], 9 * 4 * EMB * esz)
            w2c_sb = const_pool.tile([1, NUM_COMMANDS, EMB], MMD)
            dma(w2c_sb[:], w2_d[:].rearrange("e m -> (e m)")[None, :], 4 * EMB * esz)
            b2tail_sb = const_pool.tile([P, T], f32)
            dma(b2tail_sb[:], b2tail_d[:], P * T * 4)

            # broadcast w2 columns across 128 partitions via K=1 matmul
            ones_sb = const_pool.tile([1, P], MMD)
            nc.vector.memset(ones_sb[:], 1.0)
            w2_sb = {}
            for e in range(NUM_COMMANDS):
                if caps[e] == 0:
                    continue
                psw = ps_pool.tile([P, EMB], f32, tag="h", name=f"psw_{e}")
                nc.tensor.matmul(
                    psw[:], lhsT=ones_sb[:], rhs=w2c_sb[:, e, :], start=True, stop=True
                )
                w2_sb[e] = w2_pool.tile([P, EMB], STO, tag="w2", name=f"w2_sb_{e}")
                nc.scalar.activation(
                    w2_sb[e][:], psw[:], mybir.ActivationFunctionType.Copy
                )

            p_all = out_pool.tile([P, T], f32)

            A_sb = {}
            img_sb = {}
            for e, cap in enumerate(caps):
                if cap == 0:
                    continue
                A_sb[e] = w_pool.tile([P, 4, EMB], MMD, tag="A", name=f"A_sb_{e}")
                dma(
                    A_sb[e][:],
                    A_d[e].rearrange("p (o m) -> p o m", o=4),
                    P * 4 * EMB * esz,
                )
                img_sb[e] = img_pool.tile([P, 4, cap], MMD, tag="img", name=f"img_sb_{e}")
                base = 4 * sum(caps[:e])
                dma(
                    img_sb[e][:],
                    imgT_d[:, base : base + 4 * cap].rearrange("p (o r) -> p o r", o=4),
                    P * 4 * cap * esz,
                )

            g = 0
            for e, cap in enumerate(caps):
                off = sum(caps[:e])
                for r in range(cap // P):
                    ps = ps_pool.tile([P, EMB], f32, tag="h")
                    for ko in range(4):
                        nc.tensor.matmul(
                            ps[:],
                            lhsT=img_sb[e][:, ko, r * P : (r + 1) * P],
                            rhs=A_sb[e][:, ko, :],
                            start=(ko == 0),
                            stop=False,
                        )
                    col = off + r * P
                    nc.tensor.matmul(
                        ps[:],
                        lhsT=measT_sb[:, col : col + P],
                        rhs=WfAug_sb[:, e, :],
                        start=False,
                        stop=True,
                    )
                    relu_t = relu_pool.tile([P, EMB], STO, tag="relu")
                    nc.scalar.activation(
                        relu_t[:], ps[:], mybir.ActivationFunctionType.Relu
                    )
                    junk = junk_pool.tile([P, EMB], STO, tag="junk")
                    nc.vector.scalar_tensor_tensor(
                        out=junk[:],
                        in0=relu_t[:],
                        scalar=1.0,
                        in1=w2_sb[e][:],
                        op0=mybir.AluOpType.mult,
                        op1=mybir.AluOpType.mult,
                        accum_out=p_all[:, g : g + 1],
                    )
                    g += 1

            q = out_pool.tile([P, T], f32)
            nc.vector.tensor_add(q[:], p_all[:], b2tail_sb[:])
            sig = out_pool.tile([P, T], f32)
            nc.scalar.activation(
                sig[:], q[:], mybir.ActivationFunctionType.Sigmoid
            )
            outs = out_pool.tile([P, 2, T], f32)
            nc.vector.tensor_scalar_mul(outs[:, 0, :], sig[:], 50.0)
            nc.vector.tensor_scalar(
                outs[:, 1, :],
                q[:],
                1.0,
                -1.0,
                mybir.AluOpType.min,
                mybir.AluOpType.max,
            )
            nc.sync.dma_start(outp_d[:], outs[:])

    _strip_tail(nc)
    _split_excess_waits(nc)
    return nc


def _prepare(inputs, mode):
    img_embs = np.asarray(inputs["img_embs"], np.float32)
    measurements = np.asarray(inputs["measurements"], np.float32)
    command = np.asarray(inputs["command"])
    W_meas = np.asarray(inputs["W_meas"], np.float32)
    b_meas = np.asarray(inputs["b_meas"], np.float32)
    W1 = np.asarray(inputs["W1"], np.float32)
    b1 = np.asarray(inputs["b1"], np.float32)
    W2 = np.asarray(inputs["W2"], np.float32)
    b2 = np.asarray(inputs["b2"], np.float32)

    sto = _np_sto_dtype(mode)
    caps, I = _route(command)
    R = int(sum(caps))

    # fold measurement path (float64 for the host-side precompute)
    W1h = W1[:, EMB:, :].astype(np.float64)
    Wf = np.einsum("md,edh->emh", W_meas.astype(np.float64), W1h)
    b_eff = np.einsum("d,edh->eh", b_meas.astype(np.float64), W1h) + b1
    WfAug = np.concatenate([Wf, b_eff[:, None, :]], axis=1).astype(sto)  # [E,9,H]
    A = np.ascontiguousarray(W1[:, :EMB, :]).astype(sto)  # [E,512,512]
    w2c = W2[:, :, 0]
    b2c = [float(x) for x in b2[:, 0]]

    T = R // P
    col_expert = np.concatenate(
        [np.full(caps[e] // P, e, np.int64) for e in range(NUM_COMMANDS)]
    )
    b2tail = np.broadcast_to(
        np.array([b2c[e] for e in col_expert], np.float32)[None, :], (P, T)
    ).copy()

    # pre-tiled shared weights: every device DMA is a dense 2D copy
    A_pre = np.ascontiguousarray(
        A.reshape(NUM_COMMANDS, 4, P, EMB).transpose(0, 2, 1, 3).reshape(
            NUM_COMMANDS, P, 4 * EMB
        )
    )
    WfAug_pre = np.ascontiguousarray(WfAug.transpose(1, 0, 2))  # [9, E, 512]

    imgT = img_embs.T.astype(sto)  # [512, B] cast once
    measT = measurements.T  # [8, B]
    ones_row = np.ones((1, R), np.float32).astype(sto)
    in_maps = []
    for k in range(NCORES):
        Ik = I[k]
        imgT_k = imgT[:, Ik].reshape(4, P, R)  # [o, p, r]
        img_pre = np.concatenate(
            [
                imgT_k[:, :, sum(caps[:e]) : sum(caps[: e + 1])]
                .transpose(1, 0, 2)
                .reshape(P, 4 * caps[e])
                for e in range(NUM_COMMANDS)
                if caps[e]
            ],
            axis=1,
        )
        measAug_k = np.concatenate(
            [measT[:, Ik].astype(sto), ones_row], axis=0
        )
        in_maps.append(
            {
                "img_pre": np.ascontiguousarray(img_pre),
                "measAug": measAug_k,
                "A_pre": A_pre,
                "WfAug_pre": WfAug_pre,
                "w2c": np.ascontiguousarray(w2c).astype(sto),
                "b2tail": b2tail,
            }
        )
    return in_maps, I, R, caps, b2c


def _run(inputs, mode=None, trace=False):
    """Returns ((angle, speed), BassKernelResults)."""
    mode = mode or MODE
    _install_ntff_shim()
    from concourse.bass_utils import run_bass_kernel_spmd

    in_maps, I, R, caps, b2c = _prepare(inputs, mode)
    key = (R, tuple(caps), mode, tuple(np.float32(b) for b in b2c))
    if key not in _CACHE:
        _CACHE[key] = _build_program(R, caps, b2c, mode)
    nc = _CACHE[key]

    res = run_bass_kernel_spmd(
        nc, in_maps, core_ids=list(range(NCORES)), trace=trace
    )

    angle = np.zeros(B, np.float32)
    speed = np.zeros(B, np.float32)
    for k in range(NCORES):
        outp = res.results[k]["outp"]  # [128, 2, T]
        Ik = I[k]
        angle[Ik] = outp[:, 0, :].T.reshape(R)
        speed[Ik] = outp[:, 1, :].T.reshape(R)
    return (angle, speed), res


def kernel(**inputs):
    out, _ = _run(inputs)
    return out


# revision 7
# speedup vs baseline: 1.5753x; 1.0256x over previous
"""Trainium2 Bass kernel for nn_BranchedNetwork (moe_routing).

Computation (reference):
    meas_embs = measurements @ W_meas + b_meas           [B, 512]
    embs      = concat([img_embs, meas_embs], axis=1)    [B, 1024]
    h_e       = relu(embs @ W1[e] + b1[e])               per expert e
    out_e     = h_e @ W2[e] + b2[e]
    p[i]      = out[command[i], i, 0]
    angle     = sigmoid(p) * 50 ; speed = clip(p, -1, 1)

Strategy:
  * Per-sample routing is done on the host: samples are grouped by
    command id, each group padded to a multiple of 8*128 rows and
    split evenly over the 8 cores (data parallel, weights replicated).
  * Only the selected expert runs per sample (4x less compute), and
    only column 0 of W2 is needed.
  * The measurement path is folded on the host:
      h_pre = img @ W1[e][:512] + meas @ (W_meas @ W1[e][512:])
              + (b_meas @ W1[e][512:] + b1[e])
    so the device contraction is K = 512 (img) + 8 (meas) + 1 (bias
    via a ones row) instead of 1024.
  * Device per 128-row tile: 5 PE matmuls (psum [128 rows, 512 hid]),
    ACT relu psum->sbuf, then one DVE tensor_tensor_reduce computing
    p = relu_h @ w2col + b2 as a fused multiply + free-dim reduction.
    Final sigmoid/scale/clip on [128, n_tiles] at full lane width.
"""

import os
import sys
import types

import numpy as np

if "/opt/trn_rl_repo" not in sys.path and not any(
    p.endswith("trn_rl_repo") for p in sys.path
):
    sys.path.insert(0, "/opt/trn_rl_repo")

B = 16384
EMB = 512
NUM_COMMANDS = 4
NUM_MEAS = 8
NCORES = 8
P = 128

# matmul dtype mode: "f32" (exact, 4 cyc/row), "f32r" (full speed,
# reduced internal precision), "bf16" (full speed + half DMA traffic)
MODE = os.environ.get("KERNEL_MM_MODE", "f32r")

_CACHE = {}


def _install_ntff_shim():
    """Recreate antenv.axon_hooks so trace=True works if requested."""
    if "antenv.axon_hooks" in sys.modules:
        return
    try:
        import antenv

        mod = types.ModuleType("antenv.axon_hooks")
        mod._hook = None
        mod.set_axon_ntff_profile_hook = lambda h: setattr(mod, "_hook", h)
        mod.get_axon_ntff_profile_hook = lambda: mod._hook
        sys.modules["antenv.axon_hooks"] = mod
        antenv.axon_hooks = mod
        from trn_agent_boot.trn_boot import _ntff_profile_via_ctypes

        mod.set_axon_ntff_profile_hook(
            _ntff_profile_via_ctypes("/opt/axon/libaxon_pjrt.so")
        )
    except Exception:
        pass


def _split_excess_waits(nc, max_waits=1):
    """The walrus in this container rejects instructions with more than
    one embedded sync-wait command. Waits execute in order on the
    issuing engine, so hoisting the excess onto preceding NOPs on the
    same engine is semantically identical."""
    from concourse import mybir

    n_split = 0
    for f in nc.m.functions:
        for bb in f.blocks:
            insts = list(bb.instructions)
            new_insts = []
            changed = False
            for inst in insts:
                si = inst.sync_info
                if si is not None and si.on_wait and len(si.on_wait) > max_waits:
                    waits = list(si.on_wait)
                    extra, keep = waits[:-max_waits], waits[-max_waits:]
                    while extra:
                        chunk, extra = extra[:max_waits], extra[max_waits:]
                        n_split += 1
                        nop = mybir.InstNoOp(
                            name=f"waitsplit_{n_split}_{inst.name}",
                            engine=inst.engine,
                            ins=[],
                            outs=[],
                            sync_info=mybir.SyncInfo(on_wait=chunk, on_update=[]),
                        )
                        new_insts.append(nop)
                    si.on_wait = keep
                    changed = True
                new_insts.append(inst)
            if changed:
                bb.instructions.clear()
                for i in new_insts:
                    bb.instructions.append(i)
    return n_split


def _strip_tail(nc):
    """Remove the end-of-kernel barrier/sem-reset tail.

    The runtime clears semaphores in its own exec preamble, and every
    engine's results flow into the output DMA via data-dependency
    semaphores, so the only thing that must remain is the sync-engine
    DRAIN that flushes the output DMA queue."""
    from concourse import mybir

    f = nc.m.functions[0]
    bb = f.blocks[-1]
    insts = list(bb.instructions)
    idx = None
    for i, inst in enumerate(insts):
        if isinstance(inst, mybir.InstDrain) and inst.engine == mybir.EngineType.SP:
            idx = i
            break
    if idx is None:
        return 0
    kept = insts[: idx + 1]
    drain = kept[-1]
    if drain.sync_info is not None:
        drain.sync_info.on_wait = []
    removed = len(insts) - len(kept)
    bb.instructions.clear()
    for i in kept:
        bb.instructions.append(i)
    return removed


def _np_sto_dtype(mode):
    if mode == "bf16":
        import ml_dtypes

        return ml_dtypes.bfloat16
    return np.float32


def _route(command):
    """Group sample indices by expert, pad each group to a multiple of
    8*128 and split evenly across cores.

    Returns caps [E] (rows per core per expert) and I [NCORES, R] row
    index arrays (R = sum(caps))."""
    caps = []
    parts = []  # per expert: [NCORES, cap_e] padded index array
    for e in range(NUM_COMMANDS):
        idx = np.nonzero(command == e)[0].astype(np.int64)
        n = len(idx)
        cap = int(np.ceil(n / (NCORES * P))) * P if n else 0
        caps.append(cap)
        if cap == 0:
            parts.append(np.zeros((NCORES, 0), np.int64))
            continue
        pad = NCORES * cap - n
        idx_pad = np.concatenate([idx, np.full(pad, idx[-1], np.int64)])
        parts.append(idx_pad.reshape(NCORES, cap))
    I = [np.concatenate([parts[e][k] for e in range(NUM_COMMANDS)]) for k in range(NCORES)]
    return caps, np.stack(I)


def _build_program(R, caps, b2c, mode):
    from contextlib import ExitStack

    import concourse.bass as bass
    import concourse.tile as tile
    from concourse import mybir

    f32 = mybir.dt.float32
    # matmul-operand dtype (the whole producer chain must carry it for
    # the fp32r BIR verifier) and elementwise/storage dtype
    if mode == "bf16":
        MMD = mybir.dt.bfloat16
        STO = mybir.dt.bfloat16
    elif mode == "f32r":
        MMD = mybir.dt.float32r
        STO = f32
    else:
        MMD = f32
        STO = f32
    T = R // P

    nc = bass.Bass()
    # all arrays are PRE-TILED on the host so every DMA is a dense
    # [partition, contiguous-bytes] copy (cheap descriptor generation)
    imgT_d = nc.declare_dram_parameter("img_pre", [P, 4 * R], MMD, isOutput=False)
    measT_d = nc.declare_dram_parameter("measAug", [NUM_MEAS + 1, R], MMD, isOutput=False)
    A_d = nc.declare_dram_parameter("A_pre", [NUM_COMMANDS, P, 4 * EMB], MMD, isOutput=False)
    WfAug_d = nc.declare_dram_parameter(
        "WfAug_pre", [NUM_MEAS + 1, NUM_COMMANDS, EMB], MMD, isOutput=False
    )
    w2_d = nc.declare_dram_parameter("w2c", [NUM_COMMANDS, EMB], MMD, isOutput=False)
    b2tail_d = nc.declare_dram_parameter("b2tail", [P, T], f32, isOutput=False)
    outp_d = nc.declare_dram_parameter("outp", [P, 2, T], f32, isOutput=True)

    with tile.TileContext(nc) as tc:
        with ExitStack() as ctx:
            const_pool = ctx.enter_context(tc.tile_pool(name="const", bufs=1))
            w_pool = ctx.enter_context(tc.tile_pool(name="w", bufs=4))
            w2_pool = ctx.enter_context(tc.tile_pool(name="w2", bufs=4))
            img_pool = ctx.enter_context(tc.tile_pool(name="img", bufs=4))
            relu_pool = ctx.enter_context(tc.tile_pool(name="relu", bufs=6))
            junk_pool = ctx.enter_context(tc.tile_pool(name="junk", bufs=3))
            out_pool = ctx.enter_context(tc.tile_pool(name="out", bufs=1))
            ps_pool = ctx.enter_context(tc.tile_pool(name="ps", bufs=6, space="PSUM"))

            # greedy least-loaded DMA queue assignment over the three
            # DMA-capable engines (SP + ACT hwdge, Pool swdge)
            dma_engines = [nc.sync, nc.scalar, nc.gpsimd]
            dma_load = [0, 0, 0]

            def dma(dst, src, nbytes):
                qi = dma_load.index(min(dma_load))
                dma_load[qi] += nbytes
                dma_engines[qi].dma_start(dst, src)

            esz = 2 if mode == "bf16" else 4
            measT_sb = const_pool.tile([NUM_MEAS + 1, R], MMD)
            dma(measT_sb[:], measT_d[:], 9 * R * esz)
            WfAug_sb = const_pool.tile([NUM_MEAS + 1, NUM_COMMANDS, EMB], MMD)
            dma(WfAug_sb[:], WfAug_d[:], 9 * 4 * EMB * esz)
            w2c_sb = const_pool.tile([1, NUM_COMMANDS, EMB], MMD)
            dma(w2c_sb[:], w2_d[:].rearrange("e m -> (e m)")[None, :], 4 * EMB * esz)
            b2tail_sb = const_pool.tile([P, T], f32)
            dma(b2tail_sb[:], b2tail_d[:], P * T * 4)

            # broadcast w2 columns across 128 partitions via K=1 matmul
            ones_sb = const_pool.tile([1, P], MMD)
            nc.vector.memset(ones_sb[:], 1.0)
            w2_sb = {}
            for e in range(NUM_COMMANDS):
                if caps[e] == 0:
                    continue
                psw = ps_pool.tile([P, EMB], f32, tag="h", name=f"psw_{e}")
                nc.tensor.matmul(
                    psw[:], lhsT=ones_sb[:], rhs=w2c_sb[:, e, :], start=True, stop=True
                )
                w2_sb[e] = w2_pool.tile([P, EMB], STO, tag="w2", name=f"w2_sb_{e}")
                nc.scalar.activation(
                    w2_sb[e][:], psw[:], mybir.ActivationFunctionType.Copy
                )

            p_all = out_pool.tile([P, T], f32)

            A_sb = {}
            img_sb = {}
            for e, cap in enumerate(caps):
                if cap == 0:
                    continue
                A_sb[e] = w_pool.tile([P, 4, EMB], MMD, tag="A", name=f"A_sb_{e}")
                img_sb[e] = img_pool.tile([P, 4, cap], MMD, tag="img", name=f"img_sb_{e}")
                base = 4 * sum(caps[:e])
                for c in range(2):
                    dma(
                        A_sb[e][:, 2 * c : 2 * c + 2, :],
                        A_d[e][:, 2 * c * EMB : (2 * c + 2) * EMB].rearrange(
                            "p (o m) -> p o m", o=2
                        ),
                        P * 2 * EMB * esz,
                    )
                    dma(
                        img_sb[e][:, 2 * c : 2 * c + 2, :],
                        imgT_d[
                            :, base + 2 * c * cap : base + (2 * c + 2) * cap
                        ].rearrange("p (o r) -> p o r", o=2),
                        P * 2 * cap * esz,
                    )

            g = 0
            for e, cap in enumerate(caps):
                off = sum(caps[:e])
                for r in range(cap // P):
                    ps = ps_pool.tile([P, EMB], f32, tag="h")
                    for ko in range(4):
                        nc.tensor.matmul(
                            ps[:],
                            lhsT=img_sb[e][:, ko, r * P : (r + 1) * P],
                            rhs=A_sb[e][:, ko, :],
                            start=(ko == 0),
                            stop=False,
                        )
                    col = off + r * P
                    nc.tensor.matmul(
                        ps[:],
                        lhsT=measT_sb[:, col : col + P],
                        rhs=WfAug_sb[:, e, :],
                        start=False,
                        stop=True,
                    )
                    relu_t = relu_pool.tile([P, EMB], STO, tag="relu")
                    nc.scalar.activation(
                        relu_t[:], ps[:], mybir.ActivationFunctionType.Relu
                    )
                    junk = junk_pool.tile([P, EMB], STO, tag="junk")
                    nc.vector.scalar_tensor_tensor(
                        out=junk[:],
                        in0=relu_t[:],
                        scalar=1.0,
                        in1=w2_sb[e][:],
                        op0=mybir.AluOpType.mult,
                        op1=mybir.AluOpType.mult,
                        accum_out=p_all[:, g : g + 1],
                    )
                    g += 1

            q = out_pool.tile([P, T], f32)
            sig = out_pool.tile([P, T], f32)
            outs = out_pool.tile([P, 2, T], f32)
            g0 = 0
            for e, cap in enumerate(caps):
                if cap == 0:
                    continue
                g1 = g0 + cap // P
                seg = slice(g0, g1)
                nc.vector.tensor_add(q[:, seg], p_all[:, seg], b2tail_sb[:, seg])
                nc.scalar.activation(
                    sig[:, seg], q[:, seg], mybir.ActivationFunctionType.Sigmoid
                )
                nc.vector.tensor_scalar_mul(outs[:, 0, seg], sig[:, seg], 50.0)
                nc.vector.tensor_scalar(
                    outs[:, 1, seg],
                    q[:, seg],
                    1.0,
                    -1.0,
                    mybir.AluOpType.min,
                    mybir.AluOpType.max,
                )
                g0 = g1
            nc.sync.dma_start(outp_d[:], outs[:])

    _strip_tail(nc)
    _split_excess_waits(nc)
    return nc


def _prepare(inputs, mode):
    img_embs = np.asarray(inputs["img_embs"], np.float32)
    measurements = np.asarray(inputs["measurements"], np.float32)
    command = np.asarray(inputs["command"])
    W_meas = np.asarray(inputs["W_meas"], np.float32)
    b_meas = np.asarray(inputs["b_meas"], np.float32)
    W1 = np.asarray(inputs["W1"], np.float32)
    b1 = np.asarray(inputs["b1"], np.float32)
    W2 = np.asarray(inputs["W2"], np.float32)
    b2 = np.asarray(inputs["b2"], np.float32)

    sto = _np_sto_dtype(mode)
    caps, I = _route(command)
    R = int(sum(caps))

    # fold measurement path (float64 for the host-side precompute)
    W1h = W1[:, EMB:, :].astype(np.float64)
    Wf = np.einsum("md,edh->emh", W_meas.astype(np.float64), W1h)
    b_eff = np.einsum("d,edh->eh", b_meas.astype(np.float64), W1h) + b1
    WfAug = np.concatenate([Wf, b_eff[:, None, :]], axis=1).astype(sto)  # [E,9,H]
    A = np.ascontiguousarray(W1[:, :EMB, :]).astype(sto)  # [E,512,512]
    w2c = W2[:, :, 0]
    b2c = [float(x) for x in b2[:, 0]]

    T = R // P
    col_expert = np.concatenate(
        [np.full(caps[e] // P, e, np.int64) for e in range(NUM_COMMANDS)]
    )
    b2tail = np.broadcast_to(
        np.array([b2c[e] for e in col_expert], np.float32)[None, :], (P, T)
    ).copy()

    # pre-tiled shared weights: every device DMA is a dense 2D copy
    A_pre = np.ascontiguousarray(
        A.reshape(NUM_COMMANDS, 4, P, EMB).transpose(0, 2, 1, 3).reshape(
            NUM_COMMANDS, P, 4 * EMB
        )
    )
    WfAug_pre = np.ascontiguousarray(WfAug.transpose(1, 0, 2))  # [9, E, 512]

    imgT = img_embs.T.astype(sto)  # [512, B] cast once
    measT = measurements.T  # [8, B]
    ones_row = np.ones((1, R), np.float32).astype(sto)
    in_maps = []
    for k in range(NCORES):
        Ik = I[k]
        imgT_k = imgT[:, Ik].reshape(4, P, R)  # [o, p, r]
        img_pre = np.concatenate(
            [
                imgT_k[:, :, sum(caps[:e]) : sum(caps[: e + 1])]
                .transpose(1, 0, 2)
                .reshape(P, 4 * caps[e])
                for e in range(NUM_COMMANDS)
                if caps[e]
            ],
            axis=1,
        )
        measAug_k = np.concatenate(
            [measT[:, Ik].astype(sto), ones_row], axis=0
        )
        in_maps.append(
            {
                "img_pre": np.ascontiguousarray(img_pre),
                "measAug": measAug_k,
                "A_pre": A_pre,
                "WfAug_pre": WfAug_pre,
                "w2c": np.ascontiguousarray(w2c).astype(sto),
                "b2tail": b2tail,
            }
        )
    return in_maps, I, R, caps, b2c


def _run(inputs, mode=None, trace=False):
    """Returns ((angle, speed), BassKernelResults)."""
    mode = mode or MODE
    _install_ntff_shim()
    from concourse.bass_utils import run_bass_kernel_spmd

    in_maps, I, R, caps, b2c = _prepare(inputs, mode)
    key = (R, tuple(caps), mode, tuple(np.float32(b) for b in b2c))
    if key not in _CACHE:
        _CACHE[key] = _build_program(R, caps, b2c, mode)
    nc = _CACHE[key]

    res = run_bass_kernel_spmd(
        nc, in_maps, core_ids=list(range(NCORES)), trace=trace
    )

    angle = np.zeros(B, np.float32)
    speed = np.zeros(B, np.float32)
    for k in range(NCORES):
        outp = res.results[k]["outp"]  # [128, 2, T]
        Ik = I[k]
        angle[Ik] = outp[:, 0, :].T.reshape(R)
        speed[Ik] = outp[:, 1, :].T.reshape(R)
    return (angle, speed), res


def kernel(**inputs):
    out, _ = _run(inputs)
    return out
